# revision 27
# baseline (speedup 1.0000x reference)
"""CorrelationFusion Trainium2 kernel (v3).

Per-clip math (T=8 frames, G=4 groups, 3x3 correlation window):
  corr[g, tt*9+ij, p] = sum_cp x[tt, g*64+cp, p] * xpad[tt+1, g*64+cp, p+d(ij)]
  wx[g, o*8+t, p]     = sum_i conv_w[g, o*8+t, i]*corr[g, i, p] + conv_b[g, o*8+t]
  out[o, g*64+cp, p]  = sum_t wx'[g, o*8+t, p] * x[t, cp*4+g, p]
  (wx' = wx + 1 on the t==o rows -- the residual folded into the conv bias)

Mapping highlights:
  - per-pixel products on VectorE in bf16 (2x mode); the three di-shifts of a
    dj-class are fused into one big tensor_tensor via an overlapping-window AP
    (saves the fixed per-op drain tail); phase-3 fuses the four cpc products
    via a stride-0 broadcast AP on the replicated wx operand
  - the 1x1 grouped conv is FUSED into the partition-reduction matmul: lhsT
    for product r is the conv weight column replicated down the 64 cp rows of
    each group, accumulated over r in PSUM so wx emerges directly
  - 128-wide bf16 lhsT everywhere keeps fast-weight-load enabled
  - replication pads on ScalarE (GpSimd would lock the shared SBUF port that
    VectorE's second operand read needs)
  - outputs drain PSUM->SBUF as bf16 and ship bf16; host upcasts to fp32
  - data-parallel over the 8 clips: one clip per NeuronCore
"""

import numpy as np
import ml_dtypes

T = 8
TO = 8
G = 4
CPG = 64
C = 256
H = 56
W = 56
PIX = H * W
NCORES = 8
PH = 58   # padded tile rows
PW = 60   # padded tile cols (extra pad for 4B alignment of bf16 rows)
NCH = 7   # pixel chunks per image
CHN = 448  # pixels per chunk
# dj-class per fused product group: b-tile and column start within it
DJ_GROUPS = [0, 1, -1]

_CACHE = {}


def _build_module():
    import concourse.bass as bass
    import concourse.bacc as bacc
    import concourse.mybir as mybir
    import concourse.tile as tile

    fp32 = mybir.dt.float32
    bf16 = mybir.dt.bfloat16
    int32 = mybir.dt.int32

    nc = bacc.Bacc(name="corrfusion")
    xin = nc.dram_tensor("xin", [T, C, H, W], bf16, kind="ExternalInput")
    wfc = nc.dram_tensor("wfc", [2, 2, 63, 128], bf16, kind="ExternalInput")
    tones = nc.dram_tensor("tones", [128, 4, 128], bf16, kind="ExternalInput")
    bvec = nc.dram_tensor("bvec", [128, 2], fp32, kind="ExternalInput")
    out = nc.dram_tensor("out", [TO, C, H, W], bf16, kind="ExternalOutput")

    wfc_base = wfc[:, :, :, :]
    xin_flat = xin.rearrange("t c h w -> t c (h w)")          # [8, 256, 3136]
    xin_base = xin[:, :, :, :]                                 # base AP for manual APs
    out_r = out.rearrange("o (g cp) h w -> o g cp (h w)", g=4, cp=64)

    with tile.TileContext(nc) as tc:
        with tc.tile_pool(name="consts", bufs=1) as consts:

            wf_sb = consts.tile([128, 2, 63, 128], bf16)
            to_sb = consts.tile([128, 4, 128], bf16)
            bv_sb = consts.tile([128, 2], fp32)

            def load_consts(ct):
                # deferred so the first frame DMAs win the queue race; the
                # conv lhsT is cp-replicated, so expand the compact [2,2,63,128]
                # host tensor down each 64-row block via a partition-stride-0
                # broadcast DMA (64x) instead of shipping 4MB of zeros
                if ct == 0:
                    for gh in range(2):
                        bsrc = bass.AP(
                            tensor=wfc_base.tensor,
                            offset=gh * (2 * 63 * 128),
                            ap=[[0, 64], [63 * 128, 2], [128, 63], [1, 128]],
                        )
                        nc.sync.dma_start(
                            out=wf_sb[gh * 64:(gh + 1) * 64, :, :, :], in_=bsrc
                        )
                    nc.sync.dma_start(out=to_sb, in_=tones[:, :, :])
                    nc.sync.dma_start(out=bv_sb, in_=bvec[:, :])

            _build_body(nc, tc, tile, bass, mybir, fp32, bf16, int32, xin_flat,
                        xin_base, out_r, wf_sb, to_sb, bv_sb, load_consts)
    nc.compile()
    return nc


def _overlap3(bass, X, colstart):
    """[128, 3, 56, 56] view of padded tile X: dim1 = di window (rows 0..2+55)."""
    full = X[:, :, :]
    return bass.AP(
        tensor=full.tensor,
        offset=full.offset + colstart,
        ap=[list(full.ap[0]), [PW, 3], [PW, 56], [1, 56]],
    )


def _build_body(nc, tc, tile, bass, mybir, fp32, bf16, int32, xin_flat,
                xin_base, out_r, wf_sb, to_sb, bv_sb, load_consts):
    G = 4

    def load_xt4(xt4, g):
        # partition = (cpk, t): channel c = cpc*64 + cpk*4 + g,
        # 4 cpc blocks side by side in one wide tile
        for cpc in range(4):
            src = bass.AP(
                tensor=xin_base.tensor,
                offset=(cpc * 64 + g) * PIX,
                ap=[[4 * PIX, 16], [C * PIX, T], [1, PIX]],
            )
            nc.sync.dma_start(out=xt4[:, cpc * PIX:(cpc + 1) * PIX], in_=src)

    # ---------------- correlation + fused conv phase ----------------
    with tc.tile_pool(name="wxdp", bufs=1, space="DRAM") as wxdp, \
         tc.tile_pool(name="xt0p", bufs=1) as xt0p:
      wxd = []
      # g=0's gather lives in its own pool that coexists with the phase-1
      # pools, so the load isn't WAR-blocked behind the last phase-1 reader
      xt4_g0 = xt0p.tile([128, 4 * PIX], bf16, tag="xt0", name="xt4_g0")
      pr2_0 = xt0p.tile([128, 4 * PIX], bf16, tag="pr20", name="pr2_0")
      with tc.tile_pool(name="frames", bufs=1) as frames, \
           tc.tile_pool(name="prods", bufs=3) as prods, \
           tc.tile_pool(name="stage", bufs=1) as stage, \
           tc.tile_pool(name="wxbuf", bufs=1) as wxbuf, \
           tc.tile_pool(name="wxps", bufs=1, space="PSUM") as wxps:
        wx_one = wxbuf.tile([128, PIX], bf16, tag="wx", name="wx_one")
        wx_sb = [wx_one, wx_one]
        for ct in range(2):
            wps = [
                wxps.tile([128, CHN], fp32, tag=f"wp{c}", name=f"wps{ct}_{c}")
                for c in range(NCH)
            ]
            ptile = {}
            stile = {}

            def load_frame(t, ct=ct, ptile=ptile, stile=stile):
                if ct == 0 and t < 2:
                    stg = pr2_0[:, t * PIX:(t + 1) * PIX]
                else:
                    stg = stage.tile([128, PIX], bf16, tag="fstage", name="stg")
                nc.sync.dma_start(out=stg, in_=xin_flat[t, ct * 128:(ct + 1) * 128, :])
                stg3 = stg.rearrange("p (h w) -> p h w", h=H)
                P = frames.tile([128, PH, PW], bf16, tag=f"P{t % 4}", name=f"P{ct}_{t}")
                S = frames.tile([128, PH, PW], bf16, tag=f"S{t % 4}", name=f"S{ct}_{t}")
                # P fully ready first (dj=0 products need only P), then S.
                # Frame 0 is only ever the `a` center operand: skip its pads
                # and its S twin entirely.  The very first frames prep on the
                # still-idle VectorE (4x copy mode) to shorten kernel startup.
                eng_copy = (nc.vector.tensor_copy if (ct == 0 and t < 2)
                            else nc.scalar.copy)
                eng_copy(P[:, 1:57, 2:58], stg3)
                if t > 0:
                    eng_copy(P[:, 1:57, 1:2], P[:, 1:57, 2:3])
                    eng_copy(P[:, 1:57, 58:59], P[:, 1:57, 57:58])
                    eng_copy(P[:, 0:1, 1:59], P[:, 1:2, 1:59])
                    eng_copy(P[:, 57:58, 1:59], P[:, 56:57, 1:59])
                    eng_copy(S[:, 1:57, 1:57], stg3)
                    eng_copy(S[:, 1:57, 0:1], S[:, 1:57, 1:2])
                    eng_copy(S[:, 1:57, 57:58], S[:, 1:57, 56:57])
                    eng_copy(S[:, 0:1, 0:58], S[:, 1:2, 0:58])
                    eng_copy(S[:, 57:58, 0:58], S[:, 56:57, 0:58])
                ptile[t] = P
                stile[t] = S

            load_frame(0)
            load_frame(1)
            load_consts(ct)
            for tt in range(T - 1):
                if tt + 2 < T:
                    load_frame(tt + 2)
                if ct == 0 and tt == 2:
                    load_xt4(xt4_g0, 0)
                a = ptile[tt][:, 1:57, 2:58]
                a3 = a.unsqueeze(1).broadcast_to([128, 3, 56, 56])
                for grp, dj in enumerate(DJ_GROUPS):
                    if dj == 0:
                        b3 = _overlap3(bass, ptile[tt + 1], 2)
                    elif dj == 1:
                        b3 = _overlap3(bass, stile[tt + 1], 2)
                    else:
                        b3 = _overlap3(bass, stile[tt + 1], 0)
                    pr = prods.tile([128, 3 * PIX], bf16, tag="prod", name="pr")
                    pr4 = pr.rearrange("p (k h w) -> p k h w", k=3, h=H)
                    nc.vector.tensor_mul(pr4, a3, b3)
                    for k in range(3):
                        r = tt * 9 + grp * 3 + k
                        lhsT = wf_sb[:, ct, r, :]
                        for c in range(NCH):
                            nc.tensor.matmul(
                                wps[c],
                                lhsT,
                                pr[:, k * PIX + c * CHN:k * PIX + (c + 1) * CHN],
                                start=(r == 0),
                                stop=(r == 62),
                            )
            # drain wx (+bias +residual) to bf16 SBUF, then bounce to DRAM
            for c in range(NCH):
                nc.scalar.activation(
                    wx_sb[ct][:, c * CHN:(c + 1) * CHN],
                    wps[c],
                    mybir.ActivationFunctionType.Identity,
                    bias=bv_sb[:, ct:ct + 1],
                    scale=1.0,
                )
            wd = wxdp.tile([128, PIX], bf16, tag=f"wxd{ct}", name=f"wxd{ct}")
            nc.sync.dma_start(out=wd, in_=wx_sb[ct])
            wxd.append(wd)

      # ---------------- weighted-frame-sum phase ----------------
      with tc.tile_pool(name="xt", bufs=1) as xtp, \
           tc.tile_pool(name="wrep", bufs=3) as wrepp, \
           tc.tile_pool(name="pr2", bufs=2) as pr2p, \
           tc.tile_pool(name="xob", bufs=2) as xobp, \
           tc.tile_pool(name="xpsum", bufs=3, space="PSUM") as xps:

            for g in range(G):
                if g % 2 == 0:
                    xt4 = xt4_g0
                else:
                    xt4 = xtp.tile([128, 4 * PIX], bf16, tag="xt", name=f"xt{g}")
                if g > 0:
                    load_xt4(xt4, g)
                xt4v = xt4.rearrange("p (k n) -> p k n", k=4)
                for o in range(TO):
                    rowbase = (g % 2) * 64 + o * 8
                    wrep = wrepp.tile([128, PIX], bf16, tag="wrep", name="wrep")
                    wsrc = bass.AP(
                        tensor=wxd[g // 2].tensor,
                        offset=wxd[g // 2].offset + rowbase * PIX,
                        ap=[[0, 16], [PIX, 8], [1, PIX]],
                    )
                    nc.sync.dma_start(out=wrep, in_=wsrc)
                    if g == 0 and o == 0:
                        pr2 = pr2_0
                    else:
                        pr2 = pr2p.tile([128, 4 * PIX], bf16, tag="pr2", name="pr2")
                    last = (g == G - 1 and o == TO - 1)
                    if last:
                        for half in range(2):
                            nc.vector.tensor_mul(
                                pr2[:, half * 2 * PIX:(half + 1) * 2 * PIX]
                                .rearrange("p (k n) -> p k n", k=2),
                                xt4v[:, half * 2:(half + 1) * 2, :],
                                wrep.unsqueeze(1).broadcast_to([128, 2, PIX]),
                            )
                    else:
                        nc.vector.tensor_mul(
                            pr2.rearrange("p (k n) -> p k n", k=4),
                            xt4v,
                            wrep.unsqueeze(1).broadcast_to([128, 4, PIX]),
                        )
                    xout = xobp.tile([64, PIX], bf16, tag="xout", name="xout")
                    for c in range(NCH):
                        xop = xps.tile([128, CHN], fp32, tag="xo", name="xop")
                        for cpc in range(4):
                            nc.tensor.matmul(
                                xop,
                                to_sb[:, cpc, :],
                                pr2[:, cpc * PIX + c * CHN:cpc * PIX + (c + 1) * CHN],
                                start=(cpc == 0),
                                stop=(cpc == 3),
                            )
                        nc.scalar.copy(xout[:, c * CHN:(c + 1) * CHN], xop[0:64, :])
                        if g == G - 1 and o == TO - 1:
                            nc.sync.dma_start(
                                out=out_r[o, g, :, c * CHN:(c + 1) * CHN],
                                in_=xout[:, c * CHN:(c + 1) * CHN],
                            )
                    if not (g == G - 1 and o == TO - 1):
                        nc.sync.dma_start(out=out_r[o, g, :, :], in_=xout)


def _get_module():
    if "nc" not in _CACHE:
        _CACHE["nc"] = _build_module()
    return _CACHE["nc"]


def _consts(conv_w, conv_b):
    conv_w = np.asarray(conv_w, np.float32)
    conv_b = np.asarray(conv_b, np.float32)
    # fused conv lhsT per (group-pair, reordered corr-row r'): r' = tt*9 +
    # grp*3 + k covers (dj, di) = (DJ_GROUPS[grp], k-1), i.e. original column
    # ij = k*3 + DJ_GROUPS[grp] + 1.  wf[kk=gh*64+cp, gp, r', m] with
    # m = gh*64 + o*8 + t; weight independent of cp, block-diagonal in gh.
    # Bias (+1.0 residual when t==o) applied at the PSUM drain (bvec).
    wf = np.zeros((2, 2, 63, 128), np.float32)
    bv = np.zeros((128, 2), np.float32)
    rmap = np.zeros(63, np.int64)
    for tt in range(7):
        for grp, dj in enumerate(DJ_GROUPS):
            for k in range(3):
                rmap[tt * 9 + grp * 3 + k] = tt * 9 + k * 3 + dj + 1
    for gp in range(2):
        for gh in range(2):
            g = gp * 2 + gh
            for m in range(64):
                wf[gh, gp, :, gh * 64 + m] = conv_w[g, m][rmap]
                bv[gh * 64 + m, gp] = conv_b[g, m] + (1.0 if (m // 8) == (m % 8) else 0.0)

    # t-reduce ones: tones[p=(cpk,t), cpc, m] = 1 iff m == cpc*16 + cpk
    to = np.zeros((128, 4, 128), np.float32)
    for cpc in range(4):
        for cpk in range(16):
            to[cpk * 8:(cpk + 1) * 8, cpc, cpc * 16 + cpk] = 1.0

    return (
        wf.astype(ml_dtypes.bfloat16),
        to.astype(ml_dtypes.bfloat16),
        bv,
    )


def kernel(x, conv_w, conv_b):
    from concourse.bass_utils import run_bass_kernel_spmd

    nc = _get_module()
    wf, to, bv = _consts(conv_w, conv_b)
    x = np.asarray(x, np.float32).astype(ml_dtypes.bfloat16)
    x8 = np.ascontiguousarray(x.reshape(NCORES, T, C, H, W))
    in_maps = [
        {
            "xin": np.ascontiguousarray(x8[i]),
            "wfc": wf,
            "tones": to,
            "bvec": bv,
        }
        for i in range(NCORES)
    ]
    res = run_bass_kernel_spmd(nc, in_maps, core_ids=list(range(NCORES)))
    outs = [r["out"] for r in res.results]
    return np.concatenate(outs, axis=0).astype(np.float32)


# revision 28
# speedup vs baseline: 1.0247x; 1.0247x over previous
"""CorrelationFusion Trainium2 kernel (v3).

Per-clip math (T=8 frames, G=4 groups, 3x3 correlation window):
  corr[g, tt*9+ij, p] = sum_cp x[tt, g*64+cp, p] * xpad[tt+1, g*64+cp, p+d(ij)]
  wx[g, o*8+t, p]     = sum_i conv_w[g, o*8+t, i]*corr[g, i, p] + conv_b[g, o*8+t]
  out[o, g*64+cp, p]  = sum_t wx'[g, o*8+t, p] * x[t, cp*4+g, p]
  (wx' = wx + 1 on the t==o rows -- the residual folded into the conv bias)

Mapping highlights:
  - per-pixel products on VectorE in bf16 (2x mode); the three di-shifts of a
    dj-class are fused into one big tensor_tensor via an overlapping-window AP
    (saves the fixed per-op drain tail); phase-3 fuses the four cpc products
    via a stride-0 broadcast AP on the replicated wx operand
  - the 1x1 grouped conv is FUSED into the partition-reduction matmul: lhsT
    for product r is the conv weight column replicated down the 64 cp rows of
    each group, accumulated over r in PSUM so wx emerges directly
  - 128-wide bf16 lhsT everywhere keeps fast-weight-load enabled
  - replication pads on ScalarE (GpSimd would lock the shared SBUF port that
    VectorE's second operand read needs)
  - outputs drain PSUM->SBUF as bf16 and ship bf16; host upcasts to fp32
  - data-parallel over the 8 clips: one clip per NeuronCore
"""

import numpy as np
import ml_dtypes

T = 8
TO = 8
G = 4
CPG = 64
C = 256
H = 56
W = 56
PIX = H * W
NCORES = 8
PH = 58   # padded tile rows
PW = 60   # padded tile cols (extra pad for 4B alignment of bf16 rows)
NCH = 7   # pixel chunks per image
CHN = 448  # pixels per chunk
# dj-class per fused product group: b-tile and column start within it
DJ_GROUPS = [0, 1, -1]

_CACHE = {}


def _build_module():
    import concourse.bass as bass
    import concourse.bacc as bacc
    import concourse.mybir as mybir
    import concourse.tile as tile

    fp32 = mybir.dt.float32
    bf16 = mybir.dt.bfloat16
    int32 = mybir.dt.int32

    nc = bacc.Bacc(name="corrfusion")
    xin = nc.dram_tensor("xin", [T, C, H, W], bf16, kind="ExternalInput")
    wfc = nc.dram_tensor("wfc", [2, 2, 63, 128], bf16, kind="ExternalInput")
    tones = nc.dram_tensor("tones", [128, 4, 128], bf16, kind="ExternalInput")
    bvec = nc.dram_tensor("bvec", [128, 2], fp32, kind="ExternalInput")
    out = nc.dram_tensor("out", [TO, C, H, W], bf16, kind="ExternalOutput")

    wfc_base = wfc[:, :, :, :]
    xin_flat = xin.rearrange("t c h w -> t c (h w)")          # [8, 256, 3136]
    xin_base = xin[:, :, :, :]                                 # base AP for manual APs
    out_r = out.rearrange("o (g cp) h w -> o g cp (h w)", g=4, cp=64)

    with tile.TileContext(nc) as tc:
        with tc.tile_pool(name="consts", bufs=1) as consts:

            wf_sb = consts.tile([128, 2, 63, 128], bf16)
            to_sb = consts.tile([128, 4, 128], bf16)
            bv_sb = consts.tile([128, 2], fp32)

            def load_consts(ct):
                # deferred so the first frame DMAs win the queue race; the
                # conv lhsT is cp-replicated, so expand the compact [2,2,63,128]
                # host tensor down each 64-row block via a partition-stride-0
                # broadcast DMA (64x) instead of shipping 4MB of zeros.  Only
                # the group-pair this ct needs, keeping the startup DMA window
                # as light as the old per-ct full load.
                for gh in range(2):
                    bsrc = bass.AP(
                        tensor=wfc_base.tensor,
                        offset=gh * (2 * 63 * 128) + ct * (63 * 128),
                        ap=[[0, 64], [128, 63], [1, 128]],
                    )
                    nc.sync.dma_start(
                        out=wf_sb[gh * 64:(gh + 1) * 64, ct, :, :], in_=bsrc
                    )
                if ct == 0:
                    nc.sync.dma_start(out=to_sb, in_=tones[:, :, :])
                    nc.sync.dma_start(out=bv_sb, in_=bvec[:, :])

            _build_body(nc, tc, tile, bass, mybir, fp32, bf16, int32, xin_flat,
                        xin_base, out_r, wf_sb, to_sb, bv_sb, load_consts)
    nc.compile()
    return nc


def _overlap3(bass, X, colstart):
    """[128, 3, 56, 56] view of padded tile X: dim1 = di window (rows 0..2+55)."""
    full = X[:, :, :]
    return bass.AP(
        tensor=full.tensor,
        offset=full.offset + colstart,
        ap=[list(full.ap[0]), [PW, 3], [PW, 56], [1, 56]],
    )


def _build_body(nc, tc, tile, bass, mybir, fp32, bf16, int32, xin_flat,
                xin_base, out_r, wf_sb, to_sb, bv_sb, load_consts):
    G = 4

    def load_xt4(xt4, g):
        # partition = (cpk, t): channel c = cpc*64 + cpk*4 + g,
        # 4 cpc blocks side by side in one wide tile
        for cpc in range(4):
            src = bass.AP(
                tensor=xin_base.tensor,
                offset=(cpc * 64 + g) * PIX,
                ap=[[4 * PIX, 16], [C * PIX, T], [1, PIX]],
            )
            nc.sync.dma_start(out=xt4[:, cpc * PIX:(cpc + 1) * PIX], in_=src)

    # ---------------- correlation + fused conv phase ----------------
    with tc.tile_pool(name="wxdp", bufs=1, space="DRAM") as wxdp, \
         tc.tile_pool(name="xt0p", bufs=1) as xt0p:
      wxd = []
      # g=0's gather lives in its own pool that coexists with the phase-1
      # pools, so the load isn't WAR-blocked behind the last phase-1 reader
      xt4_g0 = xt0p.tile([128, 4 * PIX], bf16, tag="xt0", name="xt4_g0")
      pr2_0 = xt0p.tile([128, 4 * PIX], bf16, tag="pr20", name="pr2_0")
      with tc.tile_pool(name="frames", bufs=1) as frames, \
           tc.tile_pool(name="prods", bufs=3) as prods, \
           tc.tile_pool(name="stage", bufs=1) as stage, \
           tc.tile_pool(name="wxbuf", bufs=1) as wxbuf, \
           tc.tile_pool(name="wxps", bufs=1, space="PSUM") as wxps:
        wx_one = wxbuf.tile([128, PIX], bf16, tag="wx", name="wx_one")
        wx_sb = [wx_one, wx_one]
        for ct in range(2):
            wps = [
                wxps.tile([128, CHN], fp32, tag=f"wp{c}", name=f"wps{ct}_{c}")
                for c in range(NCH)
            ]
            ptile = {}
            stile = {}

            def load_frame(t, ct=ct, ptile=ptile, stile=stile):
                if ct == 0 and t < 2:
                    stg = pr2_0[:, t * PIX:(t + 1) * PIX]
                else:
                    stg = stage.tile([128, PIX], bf16, tag="fstage", name="stg")
                nc.sync.dma_start(out=stg, in_=xin_flat[t, ct * 128:(ct + 1) * 128, :])
                stg3 = stg.rearrange("p (h w) -> p h w", h=H)
                P = frames.tile([128, PH, PW], bf16, tag=f"P{t % 4}", name=f"P{ct}_{t}")
                S = frames.tile([128, PH, PW], bf16, tag=f"S{t % 4}", name=f"S{ct}_{t}")
                # P fully ready first (dj=0 products need only P), then S.
                # Frame 0 is only ever the `a` center operand: skip its pads
                # and its S twin entirely.  The very first frames prep on the
                # still-idle VectorE (4x copy mode) to shorten kernel startup.
                eng_copy = (nc.vector.tensor_copy if (ct == 0 and t < 2)
                            else nc.scalar.copy)
                eng_copy(P[:, 1:57, 2:58], stg3)
                if t > 0:
                    eng_copy(P[:, 1:57, 1:2], P[:, 1:57, 2:3])
                    eng_copy(P[:, 1:57, 58:59], P[:, 1:57, 57:58])
                    eng_copy(P[:, 0:1, 1:59], P[:, 1:2, 1:59])
                    eng_copy(P[:, 57:58, 1:59], P[:, 56:57, 1:59])
                    eng_copy(S[:, 1:57, 1:57], stg3)
                    eng_copy(S[:, 1:57, 0:1], S[:, 1:57, 1:2])
                    eng_copy(S[:, 1:57, 57:58], S[:, 1:57, 56:57])
                    eng_copy(S[:, 0:1, 0:58], S[:, 1:2, 0:58])
                    eng_copy(S[:, 57:58, 0:58], S[:, 56:57, 0:58])
                ptile[t] = P
                stile[t] = S

            load_frame(0)
            load_frame(1)
            load_consts(ct)
            for tt in range(T - 1):
                if tt + 2 < T:
                    load_frame(tt + 2)
                if ct == 0 and tt == 2:
                    load_xt4(xt4_g0, 0)
                a = ptile[tt][:, 1:57, 2:58]
                a3 = a.unsqueeze(1).broadcast_to([128, 3, 56, 56])
                for grp, dj in enumerate(DJ_GROUPS):
                    if dj == 0:
                        b3 = _overlap3(bass, ptile[tt + 1], 2)
                    elif dj == 1:
                        b3 = _overlap3(bass, stile[tt + 1], 2)
                    else:
                        b3 = _overlap3(bass, stile[tt + 1], 0)
                    pr = prods.tile([128, 3 * PIX], bf16, tag="prod", name="pr")
                    pr4 = pr.rearrange("p (k h w) -> p k h w", k=3, h=H)
                    nc.vector.tensor_mul(pr4, a3, b3)
                    for k in range(3):
                        r = tt * 9 + grp * 3 + k
                        lhsT = wf_sb[:, ct, r, :]
                        for c in range(NCH):
                            nc.tensor.matmul(
                                wps[c],
                                lhsT,
                                pr[:, k * PIX + c * CHN:k * PIX + (c + 1) * CHN],
                                start=(r == 0),
                                stop=(r == 62),
                            )
            # drain wx (+bias +residual) to bf16 SBUF, then bounce to DRAM
            for c in range(NCH):
                nc.scalar.activation(
                    wx_sb[ct][:, c * CHN:(c + 1) * CHN],
                    wps[c],
                    mybir.ActivationFunctionType.Identity,
                    bias=bv_sb[:, ct:ct + 1],
                    scale=1.0,
                )
            wd = wxdp.tile([128, PIX], bf16, tag=f"wxd{ct}", name=f"wxd{ct}")
            nc.sync.dma_start(out=wd, in_=wx_sb[ct])
            wxd.append(wd)

      # ---------------- weighted-frame-sum phase ----------------
      with tc.tile_pool(name="xt", bufs=1) as xtp, \
           tc.tile_pool(name="wrep", bufs=3) as wrepp, \
           tc.tile_pool(name="pr2", bufs=2) as pr2p, \
           tc.tile_pool(name="xob", bufs=2) as xobp, \
           tc.tile_pool(name="xpsum", bufs=3, space="PSUM") as xps:

            for g in range(G):
                if g % 2 == 0:
                    xt4 = xt4_g0
                else:
                    xt4 = xtp.tile([128, 4 * PIX], bf16, tag="xt", name=f"xt{g}")
                if g > 0:
                    load_xt4(xt4, g)
                xt4v = xt4.rearrange("p (k n) -> p k n", k=4)
                for o in range(TO):
                    rowbase = (g % 2) * 64 + o * 8
                    wrep = wrepp.tile([128, PIX], bf16, tag="wrep", name="wrep")
                    wsrc = bass.AP(
                        tensor=wxd[g // 2].tensor,
                        offset=wxd[g // 2].offset + rowbase * PIX,
                        ap=[[0, 16], [PIX, 8], [1, PIX]],
                    )
                    nc.sync.dma_start(out=wrep, in_=wsrc)
                    if g == 0 and o == 0:
                        pr2 = pr2_0
                    else:
                        pr2 = pr2p.tile([128, 4 * PIX], bf16, tag="pr2", name="pr2")
                    last = (g == G - 1 and o == TO - 1)
                    if last:
                        for half in range(2):
                            nc.vector.tensor_mul(
                                pr2[:, half * 2 * PIX:(half + 1) * 2 * PIX]
                                .rearrange("p (k n) -> p k n", k=2),
                                xt4v[:, half * 2:(half + 1) * 2, :],
                                wrep.unsqueeze(1).broadcast_to([128, 2, PIX]),
                            )
                    else:
                        nc.vector.tensor_mul(
                            pr2.rearrange("p (k n) -> p k n", k=4),
                            xt4v,
                            wrep.unsqueeze(1).broadcast_to([128, 4, PIX]),
                        )
                    xout = xobp.tile([64, PIX], bf16, tag="xout", name="xout")
                    for c in range(NCH):
                        xop = xps.tile([128, CHN], fp32, tag="xo", name="xop")
                        for cpc in range(4):
                            nc.tensor.matmul(
                                xop,
                                to_sb[:, cpc, :],
                                pr2[:, cpc * PIX + c * CHN:cpc * PIX + (c + 1) * CHN],
                                start=(cpc == 0),
                                stop=(cpc == 3),
                            )
                        nc.scalar.copy(xout[:, c * CHN:(c + 1) * CHN], xop[0:64, :])
                        if g == G - 1 and o == TO - 1:
                            nc.sync.dma_start(
                                out=out_r[o, g, :, c * CHN:(c + 1) * CHN],
                                in_=xout[:, c * CHN:(c + 1) * CHN],
                            )
                    if not (g == G - 1 and o == TO - 1):
                        nc.sync.dma_start(out=out_r[o, g, :, :], in_=xout)


def _get_module():
    if "nc" not in _CACHE:
        _CACHE["nc"] = _build_module()
    return _CACHE["nc"]


def _consts(conv_w, conv_b):
    conv_w = np.asarray(conv_w, np.float32)
    conv_b = np.asarray(conv_b, np.float32)
    # fused conv lhsT per (group-pair, reordered corr-row r'): r' = tt*9 +
    # grp*3 + k covers (dj, di) = (DJ_GROUPS[grp], k-1), i.e. original column
    # ij = k*3 + DJ_GROUPS[grp] + 1.  wf[kk=gh*64+cp, gp, r', m] with
    # m = gh*64 + o*8 + t; weight independent of cp, block-diagonal in gh.
    # Bias (+1.0 residual when t==o) applied at the PSUM drain (bvec).
    wf = np.zeros((2, 2, 63, 128), np.float32)
    bv = np.zeros((128, 2), np.float32)
    rmap = np.zeros(63, np.int64)
    for tt in range(7):
        for grp, dj in enumerate(DJ_GROUPS):
            for k in range(3):
                rmap[tt * 9 + grp * 3 + k] = tt * 9 + k * 3 + dj + 1
    for gp in range(2):
        for gh in range(2):
            g = gp * 2 + gh
            for m in range(64):
                wf[gh, gp, :, gh * 64 + m] = conv_w[g, m][rmap]
                bv[gh * 64 + m, gp] = conv_b[g, m] + (1.0 if (m // 8) == (m % 8) else 0.0)

    # t-reduce ones: tones[p=(cpk,t), cpc, m] = 1 iff m == cpc*16 + cpk
    to = np.zeros((128, 4, 128), np.float32)
    for cpc in range(4):
        for cpk in range(16):
            to[cpk * 8:(cpk + 1) * 8, cpc, cpc * 16 + cpk] = 1.0

    return (
        wf.astype(ml_dtypes.bfloat16),
        to.astype(ml_dtypes.bfloat16),
        bv,
    )


def kernel(x, conv_w, conv_b):
    from concourse.bass_utils import run_bass_kernel_spmd

    nc = _get_module()
    wf, to, bv = _consts(conv_w, conv_b)
    x = np.asarray(x, np.float32).astype(ml_dtypes.bfloat16)
    x8 = np.ascontiguousarray(x.reshape(NCORES, T, C, H, W))
    in_maps = [
        {
            "xin": np.ascontiguousarray(x8[i]),
            "wfc": wf,
            "tones": to,
            "bvec": bv,
        }
        for i in range(NCORES)
    ]
    res = run_bass_kernel_spmd(nc, in_maps, core_ids=list(range(NCORES)))
    outs = [r["out"] for r in res.results]
    return np.concatenate(outs, axis=0).astype(np.float32)


# revision 29
# speedup vs baseline: 1.0310x; 1.0061x over previous
"""CorrelationFusion Trainium2 kernel (v3).

Per-clip math (T=8 frames, G=4 groups, 3x3 correlation window):
  corr[g, tt*9+ij, p] = sum_cp x[tt, g*64+cp, p] * xpad[tt+1, g*64+cp, p+d(ij)]
  wx[g, o*8+t, p]     = sum_i conv_w[g, o*8+t, i]*corr[g, i, p] + conv_b[g, o*8+t]
  out[o, g*64+cp, p]  = sum_t wx'[g, o*8+t, p] * x[t, cp*4+g, p]
  (wx' = wx + 1 on the t==o rows -- the residual folded into the conv bias)

Mapping highlights:
  - per-pixel products on VectorE in bf16 (2x mode); the three di-shifts of a
    dj-class are fused into one big tensor_tensor via an overlapping-window AP
    (saves the fixed per-op drain tail); phase-3 fuses the four cpc products
    via a stride-0 broadcast AP on the replicated wx operand
  - the 1x1 grouped conv is FUSED into the partition-reduction matmul: lhsT
    for product r is the conv weight column replicated down the 64 cp rows of
    each group, accumulated over r in PSUM so wx emerges directly
  - 128-wide bf16 lhsT everywhere keeps fast-weight-load enabled
  - replication pads on ScalarE (GpSimd would lock the shared SBUF port that
    VectorE's second operand read needs)
  - outputs drain PSUM->SBUF as bf16 and ship bf16; host upcasts to fp32
  - data-parallel over the 8 clips: one clip per NeuronCore
"""

import numpy as np
import ml_dtypes

T = 8
TO = 8
G = 4
CPG = 64
C = 256
H = 56
W = 56
PIX = H * W
NCORES = 8
PH = 58   # padded tile rows
PW = 60   # padded tile cols (extra pad for 4B alignment of bf16 rows)
NCH = 7   # pixel chunks per image
CHN = 448  # pixels per chunk
# dj-class per fused product group: b-tile and column start within it
DJ_GROUPS = [0, 1, -1]

_CACHE = {}


def _build_module():
    import concourse.bass as bass
    import concourse.bacc as bacc
    import concourse.mybir as mybir
    import concourse.tile as tile

    fp32 = mybir.dt.float32
    bf16 = mybir.dt.bfloat16
    int32 = mybir.dt.int32

    nc = bacc.Bacc(name="corrfusion")
    xin = nc.dram_tensor("xin", [T, C, H, W], bf16, kind="ExternalInput")
    wfin = nc.dram_tensor("wfin", [128, 2, 63, 128], bf16, kind="ExternalInput")
    tones = nc.dram_tensor("tones", [128, 4, 128], bf16, kind="ExternalInput")
    bvec = nc.dram_tensor("bvec", [128, 2], fp32, kind="ExternalInput")
    out = nc.dram_tensor("out", [TO, C, H, W], bf16, kind="ExternalOutput")

    xin_flat = xin.rearrange("t c h w -> t c (h w)")          # [8, 256, 3136]
    xin_base = xin[:, :, :, :]                                 # base AP for manual APs
    out_r = out.rearrange("o (g cp) h w -> o g cp (h w)", g=4, cp=64)

    with tile.TileContext(nc) as tc:
        with tc.tile_pool(name="consts", bufs=1) as consts:

            wf_sb = consts.tile([128, 2, 63, 128], bf16)
            to_sb = consts.tile([128, 4, 128], bf16)
            bv_sb = consts.tile([128, 2], fp32)

            def load_consts(ct):
                # deferred so the first frame DMAs win the queue race
                nc.sync.dma_start(out=wf_sb[:, ct, :, :], in_=wfin[:, ct, :, :])
                if ct == 0:
                    nc.sync.dma_start(out=to_sb, in_=tones[:, :, :])
                    nc.sync.dma_start(out=bv_sb, in_=bvec[:, :])

            _build_body(nc, tc, tile, bass, mybir, fp32, bf16, int32, xin_flat,
                        xin_base, out_r, wf_sb, to_sb, bv_sb, load_consts)
    nc.compile()
    return nc


def _overlap3(bass, X, colstart):
    """[128, 3, 56, 56] view of padded tile X: dim1 = di window (rows 0..2+55)."""
    full = X[:, :, :]
    return bass.AP(
        tensor=full.tensor,
        offset=full.offset + colstart,
        ap=[list(full.ap[0]), [PW, 3], [PW, 56], [1, 56]],
    )


def _build_body(nc, tc, tile, bass, mybir, fp32, bf16, int32, xin_flat,
                xin_base, out_r, wf_sb, to_sb, bv_sb, load_consts):
    G = 4

    def load_xt4(xt4, g):
        # partition = (cpk, t): channel c = cpc*64 + cpk*4 + g,
        # 4 cpc blocks side by side in one wide tile
        for cpc in range(4):
            src = bass.AP(
                tensor=xin_base.tensor,
                offset=(cpc * 64 + g) * PIX,
                ap=[[4 * PIX, 16], [C * PIX, T], [1, PIX]],
            )
            nc.sync.dma_start(out=xt4[:, cpc * PIX:(cpc + 1) * PIX], in_=src)

    # ---------------- correlation + fused conv phase ----------------
    with tc.tile_pool(name="wxdp", bufs=1, space="DRAM") as wxdp, \
         tc.tile_pool(name="xt0p", bufs=1) as xt0p:
      wxd = []
      # g=0's gather lives in its own pool that coexists with the phase-1
      # pools, so the load isn't WAR-blocked behind the last phase-1 reader
      xt4_g0 = xt0p.tile([128, 4 * PIX], bf16, tag="xt0", name="xt4_g0")
      pr2_0 = xt0p.tile([128, 4 * PIX], bf16, tag="pr20", name="pr2_0")
      with tc.tile_pool(name="frames", bufs=1) as frames, \
           tc.tile_pool(name="prods", bufs=3) as prods, \
           tc.tile_pool(name="stage", bufs=1) as stage, \
           tc.tile_pool(name="wxbuf", bufs=1) as wxbuf, \
           tc.tile_pool(name="wxps", bufs=1, space="PSUM") as wxps:
        wx_one = wxbuf.tile([128, PIX], bf16, tag="wx", name="wx_one")
        wx_sb = [wx_one, wx_one]
        for ct in range(2):
            wps = [
                wxps.tile([128, CHN], fp32, tag=f"wp{c}", name=f"wps{ct}_{c}")
                for c in range(NCH)
            ]
            ptile = {}
            stile = {}

            def load_frame(t, ct=ct, ptile=ptile, stile=stile):
                if ct == 0 and t < 2:
                    stg = pr2_0[:, t * PIX:(t + 1) * PIX]
                else:
                    stg = stage.tile([128, PIX], bf16, tag="fstage", name="stg")
                nc.sync.dma_start(out=stg, in_=xin_flat[t, ct * 128:(ct + 1) * 128, :])
                stg3 = stg.rearrange("p (h w) -> p h w", h=H)
                P = frames.tile([128, PH, PW], bf16, tag=f"P{t % 4}", name=f"P{ct}_{t}")
                S = frames.tile([128, PH, PW], bf16, tag=f"S{t % 4}", name=f"S{ct}_{t}")
                # P fully ready first (dj=0 products need only P), then S.
                # Frame 0 is only ever the `a` center operand: skip its pads
                # and its S twin entirely.  The very first frames prep on the
                # still-idle VectorE (4x copy mode) to shorten kernel startup.
                eng_copy = (nc.vector.tensor_copy if (ct == 0 and t < 2)
                            else nc.scalar.copy)
                eng_copy(P[:, 1:57, 2:58], stg3)
                if t > 0:
                    eng_copy(P[:, 1:57, 1:2], P[:, 1:57, 2:3])
                    eng_copy(P[:, 1:57, 58:59], P[:, 1:57, 57:58])
                    eng_copy(P[:, 0:1, 1:59], P[:, 1:2, 1:59])
                    eng_copy(P[:, 57:58, 1:59], P[:, 56:57, 1:59])
                    eng_copy(S[:, 1:57, 1:57], stg3)
                    eng_copy(S[:, 1:57, 0:1], S[:, 1:57, 1:2])
                    eng_copy(S[:, 1:57, 57:58], S[:, 1:57, 56:57])
                    eng_copy(S[:, 0:1, 0:58], S[:, 1:2, 0:58])
                    eng_copy(S[:, 57:58, 0:58], S[:, 56:57, 0:58])
                ptile[t] = P
                stile[t] = S

            load_frame(0)
            load_frame(1)
            load_consts(ct)
            for tt in range(T - 1):
                if tt + 2 < T:
                    load_frame(tt + 2)
                if ct == 0 and tt == 2:
                    load_xt4(xt4_g0, 0)
                a = ptile[tt][:, 1:57, 2:58]
                a3 = a.unsqueeze(1).broadcast_to([128, 3, 56, 56])
                for grp, dj in enumerate(DJ_GROUPS):
                    if dj == 0:
                        b3 = _overlap3(bass, ptile[tt + 1], 2)
                    elif dj == 1:
                        b3 = _overlap3(bass, stile[tt + 1], 2)
                    else:
                        b3 = _overlap3(bass, stile[tt + 1], 0)
                    pr = prods.tile([128, 3 * PIX], bf16, tag="prod", name="pr")
                    pr4 = pr.rearrange("p (k h w) -> p k h w", k=3, h=H)
                    nc.vector.tensor_mul(pr4, a3, b3)
                    for k in range(3):
                        r = tt * 9 + grp * 3 + k
                        lhsT = wf_sb[:, ct, r, :]
                        for c in range(NCH):
                            nc.tensor.matmul(
                                wps[c],
                                lhsT,
                                pr[:, k * PIX + c * CHN:k * PIX + (c + 1) * CHN],
                                start=(r == 0),
                                stop=(r == 62),
                            )
            # drain wx (+bias +residual) to bf16 SBUF, then bounce to DRAM
            for c in range(NCH):
                nc.scalar.activation(
                    wx_sb[ct][:, c * CHN:(c + 1) * CHN],
                    wps[c],
                    mybir.ActivationFunctionType.Identity,
                    bias=bv_sb[:, ct:ct + 1],
                    scale=1.0,
                )
            wd = wxdp.tile([128, PIX], bf16, tag=f"wxd{ct}", name=f"wxd{ct}")
            nc.sync.dma_start(out=wd, in_=wx_sb[ct])
            wxd.append(wd)

      # ---------------- weighted-frame-sum phase ----------------
      with tc.tile_pool(name="xt", bufs=1) as xtp, \
           tc.tile_pool(name="wrep", bufs=3) as wrepp, \
           tc.tile_pool(name="pr2", bufs=2) as pr2p, \
           tc.tile_pool(name="xob", bufs=2) as xobp, \
           tc.tile_pool(name="xpsum", bufs=3, space="PSUM") as xps:

            for g in range(G):
                if g % 2 == 0:
                    xt4 = xt4_g0
                else:
                    xt4 = xtp.tile([128, 4 * PIX], bf16, tag="xt", name=f"xt{g}")
                if g > 0:
                    load_xt4(xt4, g)
                xt4v = xt4.rearrange("p (k n) -> p k n", k=4)
                for o in range(TO):
                    rowbase = (g % 2) * 64 + o * 8
                    wrep = wrepp.tile([128, PIX], bf16, tag="wrep", name="wrep")
                    wsrc = bass.AP(
                        tensor=wxd[g // 2].tensor,
                        offset=wxd[g // 2].offset + rowbase * PIX,
                        ap=[[0, 16], [PIX, 8], [1, PIX]],
                    )
                    nc.sync.dma_start(out=wrep, in_=wsrc)
                    if g == 0 and o == 0:
                        pr2 = pr2_0
                    else:
                        pr2 = pr2p.tile([128, 4 * PIX], bf16, tag="pr2", name="pr2")
                    last = (g == G - 1 and o == TO - 1)
                    if last:
                        for half in range(2):
                            nc.vector.tensor_mul(
                                pr2[:, half * 2 * PIX:(half + 1) * 2 * PIX]
                                .rearrange("p (k n) -> p k n", k=2),
                                xt4v[:, half * 2:(half + 1) * 2, :],
                                wrep.unsqueeze(1).broadcast_to([128, 2, PIX]),
                            )
                    else:
                        nc.vector.tensor_mul(
                            pr2.rearrange("p (k n) -> p k n", k=4),
                            xt4v,
                            wrep.unsqueeze(1).broadcast_to([128, 4, PIX]),
                        )
                    xout = xobp.tile([64, PIX], bf16, tag="xout", name="xout")
                    for c in range(NCH):
                        xop = xps.tile([128, CHN], fp32, tag="xo", name="xop")
                        for cpc in range(4):
                            nc.tensor.matmul(
                                xop,
                                to_sb[:, cpc, :],
                                pr2[:, cpc * PIX + c * CHN:cpc * PIX + (c + 1) * CHN],
                                start=(cpc == 0),
                                stop=(cpc == 3),
                            )
                        nc.scalar.copy(xout[:, c * CHN:(c + 1) * CHN], xop[0:64, :])
                        if g == G - 1 and o == TO - 1:
                            nc.sync.dma_start(
                                out=out_r[o, g, :, c * CHN:(c + 1) * CHN],
                                in_=xout[:, c * CHN:(c + 1) * CHN],
                            )
                    if not (g == G - 1 and o == TO - 1):
                        nc.sync.dma_start(out=out_r[o, g, :, :], in_=xout)


def _get_module():
    if "nc" not in _CACHE:
        _CACHE["nc"] = _build_module()
    return _CACHE["nc"]


def _consts(conv_w, conv_b):
    conv_w = np.asarray(conv_w, np.float32)
    conv_b = np.asarray(conv_b, np.float32)
    # fused conv lhsT per (group-pair, reordered corr-row r'): r' = tt*9 +
    # grp*3 + k covers (dj, di) = (DJ_GROUPS[grp], k-1), i.e. original column
    # ij = k*3 + DJ_GROUPS[grp] + 1.  wf[kk=gh*64+cp, gp, r', m] with
    # m = gh*64 + o*8 + t; weight independent of cp, block-diagonal in gh.
    # Bias (+1.0 residual when t==o) applied at the PSUM drain (bvec).
    wf = np.zeros((128, 2, 63, 128), np.float32)
    bv = np.zeros((128, 2), np.float32)
    rmap = np.zeros(63, np.int64)
    for tt in range(7):
        for grp, dj in enumerate(DJ_GROUPS):
            for k in range(3):
                rmap[tt * 9 + grp * 3 + k] = tt * 9 + k * 3 + dj + 1
    for gp in range(2):
        for gh in range(2):
            g = gp * 2 + gh
            for m in range(64):
                wf[gh * 64:(gh + 1) * 64, gp, :, gh * 64 + m] = conv_w[g, m][rmap][None, :]
                bv[gh * 64 + m, gp] = conv_b[g, m] + (1.0 if (m // 8) == (m % 8) else 0.0)

    # t-reduce ones: tones[p=(cpk,t), cpc, m] = 1 iff m == cpc*16 + cpk
    to = np.zeros((128, 4, 128), np.float32)
    for cpc in range(4):
        for cpk in range(16):
            to[cpk * 8:(cpk + 1) * 8, cpc, cpc * 16 + cpk] = 1.0

    return (
        wf.astype(ml_dtypes.bfloat16),
        to.astype(ml_dtypes.bfloat16),
        bv,
    )


def kernel(x, conv_w, conv_b):
    from concourse.bass_utils import run_bass_kernel_spmd

    nc = _get_module()
    wf, to, bv = _consts(conv_w, conv_b)
    x = np.asarray(x, np.float32).astype(ml_dtypes.bfloat16)
    x8 = np.ascontiguousarray(x.reshape(NCORES, T, C, H, W))
    in_maps = [
        {
            "xin": np.ascontiguousarray(x8[i]),
            "wfin": wf,
            "tones": to,
            "bvec": bv,
        }
        for i in range(NCORES)
    ]
    res = run_bass_kernel_spmd(nc, in_maps, core_ids=list(range(NCORES)))
    outs = [r["out"] for r in res.results]
    return np.concatenate(outs, axis=0).astype(np.float32)


# revision 30
# speedup vs baseline: 1.0311x; 1.0001x over previous
"""CorrelationFusion Trainium2 kernel (v3).

Per-clip math (T=8 frames, G=4 groups, 3x3 correlation window):
  corr[g, tt*9+ij, p] = sum_cp x[tt, g*64+cp, p] * xpad[tt+1, g*64+cp, p+d(ij)]
  wx[g, o*8+t, p]     = sum_i conv_w[g, o*8+t, i]*corr[g, i, p] + conv_b[g, o*8+t]
  out[o, g*64+cp, p]  = sum_t wx'[g, o*8+t, p] * x[t, cp*4+g, p]
  (wx' = wx + 1 on the t==o rows -- the residual folded into the conv bias)

Mapping highlights:
  - per-pixel products on VectorE in bf16 (2x mode); the three di-shifts of a
    dj-class are fused into one big tensor_tensor via an overlapping-window AP
    (saves the fixed per-op drain tail); phase-3 fuses the four cpc products
    via a stride-0 broadcast AP on the replicated wx operand
  - the 1x1 grouped conv is FUSED into the partition-reduction matmul: lhsT
    for product r is the conv weight column replicated down the 64 cp rows of
    each group, accumulated over r in PSUM so wx emerges directly
  - 128-wide bf16 lhsT everywhere keeps fast-weight-load enabled
  - replication pads on ScalarE (GpSimd would lock the shared SBUF port that
    VectorE's second operand read needs)
  - outputs drain PSUM->SBUF as bf16 and ship bf16; host upcasts to fp32
  - data-parallel over the 8 clips: one clip per NeuronCore
"""

import numpy as np
import ml_dtypes

T = 8
TO = 8
G = 4
CPG = 64
C = 256
H = 56
W = 56
PIX = H * W
NCORES = 8
PH = 58   # padded tile rows
PW = 60   # padded tile cols (extra pad for 4B alignment of bf16 rows)
NCH = 7   # pixel chunks per image
CHN = 448  # pixels per chunk
# dj-class per fused product group: b-tile and column start within it
DJ_GROUPS = [0, 1, -1]

_CACHE = {}


def _build_module():
    import concourse.bass as bass
    import concourse.bacc as bacc
    import concourse.mybir as mybir
    import concourse.tile as tile

    fp32 = mybir.dt.float32
    bf16 = mybir.dt.bfloat16
    int32 = mybir.dt.int32

    nc = bacc.Bacc(name="corrfusion")
    xin = nc.dram_tensor("xin", [T, C, H, W], bf16, kind="ExternalInput")
    wfin = nc.dram_tensor("wfin", [128, 2, 63, 128], bf16, kind="ExternalInput")
    tones = nc.dram_tensor("tones", [128, 4, 128], bf16, kind="ExternalInput")
    bvec = nc.dram_tensor("bvec", [128, 2], fp32, kind="ExternalInput")
    out = nc.dram_tensor("out", [TO, C, H, W], bf16, kind="ExternalOutput")

    xin_flat = xin.rearrange("t c h w -> t c (h w)")          # [8, 256, 3136]
    xin_base = xin[:, :, :, :]                                 # base AP for manual APs
    out_r = out.rearrange("o (g cp) h w -> o g cp (h w)", g=4, cp=64)

    with tile.TileContext(nc) as tc:
        with tc.tile_pool(name="consts", bufs=1) as consts:

            wf_sb = consts.tile([128, 2, 63, 128], bf16)
            to_sb = consts.tile([128, 4, 128], bf16)
            bv_sb = consts.tile([128, 2], fp32)

            def load_consts(ct):
                # deferred so the first frame DMAs win the queue race
                nc.sync.dma_start(out=wf_sb[:, ct, :, :], in_=wfin[:, ct, :, :])
                if ct == 0:
                    nc.sync.dma_start(out=to_sb, in_=tones[:, :, :])
                    nc.sync.dma_start(out=bv_sb, in_=bvec[:, :])

            _build_body(nc, tc, tile, bass, mybir, fp32, bf16, int32, xin_flat,
                        xin_base, out_r, wf_sb, to_sb, bv_sb, load_consts)
    nc.compile()
    return nc


def _overlap3(bass, X, colstart):
    """[128, 3, 56, 56] view of padded tile X: dim1 = di window (rows 0..2+55)."""
    full = X[:, :, :]
    return bass.AP(
        tensor=full.tensor,
        offset=full.offset + colstart,
        ap=[list(full.ap[0]), [PW, 3], [PW, 56], [1, 56]],
    )


def _build_body(nc, tc, tile, bass, mybir, fp32, bf16, int32, xin_flat,
                xin_base, out_r, wf_sb, to_sb, bv_sb, load_consts):
    G = 4

    def load_xt4(xt4, g):
        # partition = (cpk, t): channel c = cpc*64 + cpk*4 + g,
        # 4 cpc blocks side by side in one wide tile
        for cpc in range(4):
            src = bass.AP(
                tensor=xin_base.tensor,
                offset=(cpc * 64 + g) * PIX,
                ap=[[4 * PIX, 16], [C * PIX, T], [1, PIX]],
            )
            nc.sync.dma_start(out=xt4[:, cpc * PIX:(cpc + 1) * PIX], in_=src)

    # ---------------- correlation + fused conv phase ----------------
    with tc.tile_pool(name="wxdp", bufs=1, space="DRAM") as wxdp, \
         tc.tile_pool(name="xt0p", bufs=1) as xt0p:
      wxd = []
      # g=0's gather lives in its own pool that coexists with the phase-1
      # pools, so the load isn't WAR-blocked behind the last phase-1 reader
      xt4_g0 = xt0p.tile([128, 4 * PIX], bf16, tag="xt0", name="xt4_g0")
      pr2_0 = xt0p.tile([128, 4 * PIX], bf16, tag="pr20", name="pr2_0")
      with tc.tile_pool(name="frames", bufs=1) as frames, \
           tc.tile_pool(name="prods", bufs=3) as prods, \
           tc.tile_pool(name="stage", bufs=1) as stage, \
           tc.tile_pool(name="wxbuf", bufs=1) as wxbuf, \
           tc.tile_pool(name="wxps", bufs=1, space="PSUM") as wxps:
        wx_one = wxbuf.tile([128, PIX], bf16, tag="wx", name="wx_one")
        wx_sb = [wx_one, wx_one]
        for ct in range(2):
            wps = [
                wxps.tile([128, CHN], fp32, tag=f"wp{c}", name=f"wps{ct}_{c}")
                for c in range(NCH)
            ]
            ptile = {}
            stile = {}

            def load_frame(t, ct=ct, ptile=ptile, stile=stile):
                if ct == 0 and t < 2:
                    stg = pr2_0[:, t * PIX:(t + 1) * PIX]
                else:
                    stg = stage.tile([128, PIX], bf16, tag="fstage", name="stg")
                nc.sync.dma_start(out=stg, in_=xin_flat[t, ct * 128:(ct + 1) * 128, :])
                stg3 = stg.rearrange("p (h w) -> p h w", h=H)
                P = frames.tile([128, PH, PW], bf16, tag=f"P{t % 4}", name=f"P{ct}_{t}")
                S = frames.tile([128, PH, PW], bf16, tag=f"S{t % 4}", name=f"S{ct}_{t}")
                # P fully ready first (dj=0 products need only P), then S.
                # Frame 0 is only ever the `a` center operand: skip its pads
                # and its S twin entirely.  The very first frames prep on the
                # still-idle VectorE (4x copy mode) to shorten kernel startup.
                eng_copy = (nc.vector.tensor_copy if (ct == 0 and t < 2)
                            else nc.scalar.copy)
                eng_copy(P[:, 1:57, 2:58], stg3)
                if t > 0:
                    eng_copy(P[:, 1:57, 1:2], P[:, 1:57, 2:3])
                    eng_copy(P[:, 1:57, 58:59], P[:, 1:57, 57:58])
                    eng_copy(P[:, 0:1, 1:59], P[:, 1:2, 1:59])
                    eng_copy(P[:, 57:58, 1:59], P[:, 56:57, 1:59])
                    eng_copy(S[:, 1:57, 1:57], stg3)
                    eng_copy(S[:, 1:57, 0:1], S[:, 1:57, 1:2])
                    eng_copy(S[:, 1:57, 57:58], S[:, 1:57, 56:57])
                    eng_copy(S[:, 0:1, 0:58], S[:, 1:2, 0:58])
                    eng_copy(S[:, 57:58, 0:58], S[:, 56:57, 0:58])
                ptile[t] = P
                stile[t] = S

            load_frame(0)
            load_frame(1)
            load_consts(ct)
            for tt in range(T - 1):
                if tt + 2 < T:
                    load_frame(tt + 2)
                if ct == 0 and tt == 2:
                    load_xt4(xt4_g0, 0)
                a = ptile[tt][:, 1:57, 2:58]
                a3 = a.unsqueeze(1).broadcast_to([128, 3, 56, 56])
                for grp, dj in enumerate(DJ_GROUPS):
                    if dj == 0:
                        b3 = _overlap3(bass, ptile[tt + 1], 2)
                    elif dj == 1:
                        b3 = _overlap3(bass, stile[tt + 1], 2)
                    else:
                        b3 = _overlap3(bass, stile[tt + 1], 0)
                    pr = prods.tile([128, 3 * PIX], bf16, tag="prod", name="pr")
                    pr4 = pr.rearrange("p (k h w) -> p k h w", k=3, h=H)
                    nc.vector.tensor_mul(pr4, a3, b3)
                    for k in range(3):
                        r = tt * 9 + grp * 3 + k
                        lhsT = wf_sb[:, ct, r, :]
                        for c in range(NCH):
                            nc.tensor.matmul(
                                wps[c],
                                lhsT,
                                pr[:, k * PIX + c * CHN:k * PIX + (c + 1) * CHN],
                                start=(r == 0),
                                stop=(r == 62),
                            )
            # drain wx (+bias +residual) to bf16 SBUF, then bounce to DRAM
            for c in range(NCH):
                nc.scalar.activation(
                    wx_sb[ct][:, c * CHN:(c + 1) * CHN],
                    wps[c],
                    mybir.ActivationFunctionType.Identity,
                    bias=bv_sb[:, ct:ct + 1],
                    scale=1.0,
                )
            wd = wxdp.tile([128, PIX], bf16, tag=f"wxd{ct}", name=f"wxd{ct}")
            nc.sync.dma_start(out=wd, in_=wx_sb[ct])
            wxd.append(wd)

      # ---------------- weighted-frame-sum phase ----------------
      with tc.tile_pool(name="xt", bufs=1) as xtp, \
           tc.tile_pool(name="wrep", bufs=3) as wrepp, \
           tc.tile_pool(name="pr2", bufs=2) as pr2p, \
           tc.tile_pool(name="xob", bufs=2) as xobp, \
           tc.tile_pool(name="xpsum", bufs=4, space="PSUM") as xps:

            for g in range(G):
                if g % 2 == 0:
                    xt4 = xt4_g0
                else:
                    xt4 = xtp.tile([128, 4 * PIX], bf16, tag="xt", name=f"xt{g}")
                if g > 0:
                    load_xt4(xt4, g)
                xt4v = xt4.rearrange("p (k n) -> p k n", k=4)
                for o in range(TO):
                    rowbase = (g % 2) * 64 + o * 8
                    wrep = wrepp.tile([128, PIX], bf16, tag="wrep", name="wrep")
                    wsrc = bass.AP(
                        tensor=wxd[g // 2].tensor,
                        offset=wxd[g // 2].offset + rowbase * PIX,
                        ap=[[0, 16], [PIX, 8], [1, PIX]],
                    )
                    nc.sync.dma_start(out=wrep, in_=wsrc)
                    if g == 0 and o == 0:
                        pr2 = pr2_0
                    else:
                        pr2 = pr2p.tile([128, 4 * PIX], bf16, tag="pr2", name="pr2")
                    last = (g == G - 1 and o == TO - 1)
                    if last:
                        for half in range(2):
                            nc.vector.tensor_mul(
                                pr2[:, half * 2 * PIX:(half + 1) * 2 * PIX]
                                .rearrange("p (k n) -> p k n", k=2),
                                xt4v[:, half * 2:(half + 1) * 2, :],
                                wrep.unsqueeze(1).broadcast_to([128, 2, PIX]),
                            )
                    else:
                        nc.vector.tensor_mul(
                            pr2.rearrange("p (k n) -> p k n", k=4),
                            xt4v,
                            wrep.unsqueeze(1).broadcast_to([128, 4, PIX]),
                        )
                    xout = xobp.tile([64, PIX], bf16, tag="xout", name="xout")
                    for c in range(NCH):
                        xop = xps.tile([128, CHN], fp32, tag="xo", name="xop")
                        for cpc in range(4):
                            nc.tensor.matmul(
                                xop,
                                to_sb[:, cpc, :],
                                pr2[:, cpc * PIX + c * CHN:cpc * PIX + (c + 1) * CHN],
                                start=(cpc == 0),
                                stop=(cpc == 3),
                            )
                        nc.scalar.copy(xout[:, c * CHN:(c + 1) * CHN], xop[0:64, :])
                        if g == G - 1 and o == TO - 1:
                            nc.sync.dma_start(
                                out=out_r[o, g, :, c * CHN:(c + 1) * CHN],
                                in_=xout[:, c * CHN:(c + 1) * CHN],
                            )
                    if not (g == G - 1 and o == TO - 1):
                        nc.sync.dma_start(out=out_r[o, g, :, :], in_=xout)


def _get_module():
    if "nc" not in _CACHE:
        _CACHE["nc"] = _build_module()
    return _CACHE["nc"]


def _consts(conv_w, conv_b):
    conv_w = np.asarray(conv_w, np.float32)
    conv_b = np.asarray(conv_b, np.float32)
    # fused conv lhsT per (group-pair, reordered corr-row r'): r' = tt*9 +
    # grp*3 + k covers (dj, di) = (DJ_GROUPS[grp], k-1), i.e. original column
    # ij = k*3 + DJ_GROUPS[grp] + 1.  wf[kk=gh*64+cp, gp, r', m] with
    # m = gh*64 + o*8 + t; weight independent of cp, block-diagonal in gh.
    # Bias (+1.0 residual when t==o) applied at the PSUM drain (bvec).
    wf = np.zeros((128, 2, 63, 128), np.float32)
    bv = np.zeros((128, 2), np.float32)
    rmap = np.zeros(63, np.int64)
    for tt in range(7):
        for grp, dj in enumerate(DJ_GROUPS):
            for k in range(3):
                rmap[tt * 9 + grp * 3 + k] = tt * 9 + k * 3 + dj + 1
    for gp in range(2):
        for gh in range(2):
            g = gp * 2 + gh
            for m in range(64):
                wf[gh * 64:(gh + 1) * 64, gp, :, gh * 64 + m] = conv_w[g, m][rmap][None, :]
                bv[gh * 64 + m, gp] = conv_b[g, m] + (1.0 if (m // 8) == (m % 8) else 0.0)

    # t-reduce ones: tones[p=(cpk,t), cpc, m] = 1 iff m == cpc*16 + cpk
    to = np.zeros((128, 4, 128), np.float32)
    for cpc in range(4):
        for cpk in range(16):
            to[cpk * 8:(cpk + 1) * 8, cpc, cpc * 16 + cpk] = 1.0

    return (
        wf.astype(ml_dtypes.bfloat16),
        to.astype(ml_dtypes.bfloat16),
        bv,
    )


def kernel(x, conv_w, conv_b):
    from concourse.bass_utils import run_bass_kernel_spmd

    nc = _get_module()
    wf, to, bv = _consts(conv_w, conv_b)
    x = np.asarray(x, np.float32).astype(ml_dtypes.bfloat16)
    x8 = np.ascontiguousarray(x.reshape(NCORES, T, C, H, W))
    in_maps = [
        {
            "xin": np.ascontiguousarray(x8[i]),
            "wfin": wf,
            "tones": to,
            "bvec": bv,
        }
        for i in range(NCORES)
    ]
    res = run_bass_kernel_spmd(nc, in_maps, core_ids=list(range(NCORES)))
    outs = [r["out"] for r in res.results]
    return np.concatenate(outs, axis=0).astype(np.float32)


# revision 32
# speedup vs baseline: 1.0395x; 1.0081x over previous
"""CorrelationFusion Trainium2 kernel (v3).

Per-clip math (T=8 frames, G=4 groups, 3x3 correlation window):
  corr[g, tt*9+ij, p] = sum_cp x[tt, g*64+cp, p] * xpad[tt+1, g*64+cp, p+d(ij)]
  wx[g, o*8+t, p]     = sum_i conv_w[g, o*8+t, i]*corr[g, i, p] + conv_b[g, o*8+t]
  out[o, g*64+cp, p]  = sum_t wx'[g, o*8+t, p] * x[t, cp*4+g, p]
  (wx' = wx + 1 on the t==o rows -- the residual folded into the conv bias)

Mapping highlights:
  - per-pixel products on VectorE in bf16 (2x mode); the three di-shifts of a
    dj-class are fused into one big tensor_tensor via an overlapping-window AP
    (saves the fixed per-op drain tail); phase-3 fuses the four cpc products
    via a stride-0 broadcast AP on the replicated wx operand
  - the 1x1 grouped conv is FUSED into the partition-reduction matmul: lhsT
    for product r is the conv weight column replicated down the 64 cp rows of
    each group, accumulated over r in PSUM so wx emerges directly
  - 128-wide bf16 lhsT everywhere keeps fast-weight-load enabled
  - replication pads on ScalarE (GpSimd would lock the shared SBUF port that
    VectorE's second operand read needs)
  - outputs drain PSUM->SBUF as bf16 and ship bf16; host upcasts to fp32
  - data-parallel over the 8 clips: one clip per NeuronCore
"""

import numpy as np
import ml_dtypes

T = 8
TO = 8
G = 4
CPG = 64
C = 256
H = 56
W = 56
PIX = H * W
NCORES = 8
PH = 58   # padded tile rows
PW = 60   # padded tile cols (extra pad for 4B alignment of bf16 rows)
NCH = 7   # pixel chunks per image
CHN = 448  # pixels per chunk
# dj-class per fused product group: b-tile and column start within it
DJ_GROUPS = [0, 1, -1]

_CACHE = {}


def _build_module():
    import concourse.bass as bass
    import concourse.bacc as bacc
    import concourse.mybir as mybir
    import concourse.tile as tile

    fp32 = mybir.dt.float32
    bf16 = mybir.dt.bfloat16
    int32 = mybir.dt.int32

    nc = bacc.Bacc(name="corrfusion")
    xin = nc.dram_tensor("xin", [T, C, H, W], bf16, kind="ExternalInput")
    wfin = nc.dram_tensor("wfin", [128, 2, 63, 128], bf16, kind="ExternalInput")
    tones = nc.dram_tensor("tones", [128, 4, 128], bf16, kind="ExternalInput")
    bvec = nc.dram_tensor("bvec", [128, 2], fp32, kind="ExternalInput")
    out = nc.dram_tensor("out", [TO, C, H, W], bf16, kind="ExternalOutput")

    xin_flat = xin.rearrange("t c h w -> t c (h w)")          # [8, 256, 3136]
    xin_base = xin[:, :, :, :]                                 # base AP for manual APs
    out_r = out.rearrange("o (g cp) h w -> o g cp (h w)", g=4, cp=64)

    with tile.TileContext(nc) as tc:
        with tc.tile_pool(name="consts", bufs=1) as consts:

            wf_sb = consts.tile([128, 2, 63, 128], bf16)
            to_sb = consts.tile([128, 4, 128], bf16)
            bv_sb = consts.tile([128, 2], fp32)

            def load_consts(ct):
                # deferred so the first frame DMAs win the queue race
                nc.sync.dma_start(out=wf_sb[:, ct, :, :], in_=wfin[:, ct, :, :])
                if ct == 0:
                    nc.sync.dma_start(out=to_sb, in_=tones[:, :, :])
                    nc.sync.dma_start(out=bv_sb, in_=bvec[:, :])

            _build_body(nc, tc, tile, bass, mybir, fp32, bf16, int32, xin_flat,
                        xin_base, out_r, wf_sb, to_sb, bv_sb, load_consts)
    nc.compile()
    return nc


def _overlap3(bass, X, colstart):
    """[128, 3, 56, 56] view of padded tile X: dim1 = di window (rows 0..2+55)."""
    full = X[:, :, :]
    return bass.AP(
        tensor=full.tensor,
        offset=full.offset + colstart,
        ap=[list(full.ap[0]), [PW, 3], [PW, 56], [1, 56]],
    )


def _build_body(nc, tc, tile, bass, mybir, fp32, bf16, int32, xin_flat,
                xin_base, out_r, wf_sb, to_sb, bv_sb, load_consts):
    G = 4

    def load_xt4(xt4, g):
        # partition = (cpk, t): channel c = cpc*64 + cpk*4 + g,
        # 4 cpc blocks side by side in one wide tile
        for cpc in range(4):
            src = bass.AP(
                tensor=xin_base.tensor,
                offset=(cpc * 64 + g) * PIX,
                ap=[[4 * PIX, 16], [C * PIX, T], [1, PIX]],
            )
            nc.sync.dma_start(out=xt4[:, cpc * PIX:(cpc + 1) * PIX], in_=src)

    # ---------------- correlation + fused conv phase ----------------
    with tc.tile_pool(name="wxdp", bufs=1, space="DRAM") as wxdp, \
         tc.tile_pool(name="xt0p", bufs=1) as xt0p:
      wxd = []
      # g=0's gather lives in its own pool that coexists with the phase-1
      # pools, so the load isn't WAR-blocked behind the last phase-1 reader
      xt4_g0 = xt0p.tile([128, 4 * PIX], bf16, tag="xt0", name="xt4_g0")
      pr2_0 = xt0p.tile([128, 4 * PIX], bf16, tag="pr20", name="pr2_0")
      with tc.tile_pool(name="frames", bufs=1) as frames, \
           tc.tile_pool(name="prods", bufs=3) as prods, \
           tc.tile_pool(name="stage", bufs=1) as stage, \
           tc.tile_pool(name="wxbuf", bufs=1) as wxbuf, \
           tc.tile_pool(name="wxps", bufs=1, space="PSUM") as wxps:
        wx_one = wxbuf.tile([128, PIX], bf16, tag="wx", name="wx_one")
        wx_sb = [wx_one, wx_one]
        for ct in range(2):
            wps = [
                wxps.tile([128, CHN], fp32, tag=f"wp{c}", name=f"wps{ct}_{c}")
                for c in range(NCH)
            ]
            ptile = {}
            stile = {}

            def load_frame(t, ct=ct, ptile=ptile, stile=stile):
                if ct == 0 and t < 2:
                    stg = pr2_0[:, t * PIX:(t + 1) * PIX]
                else:
                    stg = stage.tile([128, PIX], bf16, tag="fstage", name="stg")
                nc.sync.dma_start(out=stg, in_=xin_flat[t, ct * 128:(ct + 1) * 128, :])
                stg3 = stg.rearrange("p (h w) -> p h w", h=H)
                P = frames.tile([128, PH, PW], bf16, tag=f"P{t % 4}", name=f"P{ct}_{t}")
                S = frames.tile([128, PH, PW], bf16, tag=f"S{t % 4}", name=f"S{ct}_{t}")
                # P fully ready first (dj=0 products need only P), then S.
                # Frame 0 is only ever the `a` center operand: skip its pads
                # and its S twin entirely.  The very first frames prep on the
                # still-idle VectorE (4x copy mode) to shorten kernel startup.
                eng_copy = (nc.vector.tensor_copy if (ct == 0 and t < 2)
                            else nc.scalar.copy)
                eng_copy(P[:, 1:57, 2:58], stg3)
                if t > 0:
                    eng_copy(P[:, 1:57, 1:2], P[:, 1:57, 2:3])
                    eng_copy(P[:, 1:57, 58:59], P[:, 1:57, 57:58])
                    eng_copy(P[:, 0:1, 1:59], P[:, 1:2, 1:59])
                    eng_copy(P[:, 57:58, 1:59], P[:, 56:57, 1:59])
                    eng_copy(S[:, 1:57, 1:57], stg3)
                    eng_copy(S[:, 1:57, 0:1], S[:, 1:57, 1:2])
                    eng_copy(S[:, 1:57, 57:58], S[:, 1:57, 56:57])
                    eng_copy(S[:, 0:1, 0:58], S[:, 1:2, 0:58])
                    eng_copy(S[:, 57:58, 0:58], S[:, 56:57, 0:58])
                ptile[t] = P
                stile[t] = S

            load_frame(0)
            load_frame(1)
            load_consts(ct)
            for tt in range(T - 1):
                if tt + 2 < T:
                    load_frame(tt + 2)
                if ct == 0 and tt == 2:
                    load_xt4(xt4_g0, 0)
                a = ptile[tt][:, 1:57, 2:58]
                a3 = a.unsqueeze(1).broadcast_to([128, 3, 56, 56])
                for grp, dj in enumerate(DJ_GROUPS):
                    if dj == 0:
                        btile, cs = ptile[tt + 1], 2
                    elif dj == 1:
                        btile, cs = stile[tt + 1], 2
                    else:
                        btile, cs = stile[tt + 1], 0
                    pr = prods.tile([128, 3 * PIX], bf16, tag="prod", name="pr")
                    pr4 = pr.rearrange("p (k h w) -> p k h w", k=3, h=H)
                    if ct == 0 and tt == 0 and grp == 0:
                        # kernel-warmup split: the di=0 slice reads only the
                        # pad-free center, so it can start before the pads
                        # land; its matmuls carry the start flag
                        nc.vector.tensor_mul(
                            pr4[:, 1:2, :, :],
                            a.unsqueeze(1).broadcast_to([128, 1, 56, 56]),
                            btile[:, 1:57, 2:58].unsqueeze(1)
                            .broadcast_to([128, 1, 56, 56]),
                        )
                        full = btile[:, :, :]
                        b2 = bass.AP(
                            tensor=full.tensor,
                            offset=full.offset + cs,
                            ap=[list(full.ap[0]), [2 * PW, 2], [PW, 56], [1, 56]],
                        )
                        a2 = a.unsqueeze(1).broadcast_to([128, 2, 56, 56])
                        pr4_02 = bass.AP(
                            tensor=pr.tensor,
                            offset=pr.offset,
                            ap=[[3 * PIX, 128], [2 * PIX, 2], [56, 56], [1, 56]],
                        )
                        korder = [1, 0, 2]
                    else:
                        b3 = _overlap3(bass, btile, cs)
                        nc.vector.tensor_mul(pr4, a3, b3)
                        korder = [0, 1, 2]
                    for ki, k in enumerate(korder):
                        if ct == 0 and tt == 0 and grp == 0 and ki == 1:
                            nc.vector.tensor_mul(pr4_02, a2, b2)
                        r = tt * 9 + grp * 3 + k
                        lhsT = wf_sb[:, ct, r, :]
                        for c in range(NCH):
                            nc.tensor.matmul(
                                wps[c],
                                lhsT,
                                pr[:, k * PIX + c * CHN:k * PIX + (c + 1) * CHN],
                                start=(tt == 0 and grp == 0 and ki == 0),
                                stop=(r == 62),
                            )
            # drain wx (+bias +residual) to bf16 SBUF, then bounce to DRAM
            for c in range(NCH):
                nc.scalar.activation(
                    wx_sb[ct][:, c * CHN:(c + 1) * CHN],
                    wps[c],
                    mybir.ActivationFunctionType.Identity,
                    bias=bv_sb[:, ct:ct + 1],
                    scale=1.0,
                )
            wd = wxdp.tile([128, PIX], bf16, tag=f"wxd{ct}", name=f"wxd{ct}")
            nc.sync.dma_start(out=wd, in_=wx_sb[ct])
            wxd.append(wd)

      # ---------------- weighted-frame-sum phase ----------------
      with tc.tile_pool(name="wrep", bufs=4) as wrepp, \
           tc.tile_pool(name="pr2", bufs=2) as pr2p, \
           tc.tile_pool(name="xt", bufs=1) as xtp, \
           tc.tile_pool(name="xob", bufs=2) as xobp, \
           tc.tile_pool(name="xpsum", bufs=4, space="PSUM") as xps:

            for g in range(G):
                if g % 2 == 0:
                    xt4 = xt4_g0
                else:
                    xt4 = xtp.tile([128, 4 * PIX], bf16, tag="xt", name=f"xt{g}")
                if g > 0:
                    load_xt4(xt4, g)
                xt4v = xt4.rearrange("p (k n) -> p k n", k=4)
                for o in range(TO):
                    rowbase = (g % 2) * 64 + o * 8
                    wrep = wrepp.tile([128, PIX], bf16, tag="wrep", name="wrep")
                    wsrc = bass.AP(
                        tensor=wxd[g // 2].tensor,
                        offset=wxd[g // 2].offset + rowbase * PIX,
                        ap=[[0, 16], [PIX, 8], [1, PIX]],
                    )
                    nc.sync.dma_start(out=wrep, in_=wsrc)
                    if g == 0 and o == 0:
                        pr2 = pr2_0
                    else:
                        pr2 = pr2p.tile([128, 4 * PIX], bf16, tag="pr2", name="pr2")
                    last = (g == G - 1 and o == TO - 1)
                    if last:
                        for q in range(4):
                            nc.vector.tensor_mul(
                                pr2[:, q * PIX:(q + 1) * PIX],
                                xt4[:, q * PIX:(q + 1) * PIX],
                                wrep,
                            )
                    else:
                        nc.vector.tensor_mul(
                            pr2.rearrange("p (k n) -> p k n", k=4),
                            xt4v,
                            wrep.unsqueeze(1).broadcast_to([128, 4, PIX]),
                        )
                    xout = xobp.tile([64, PIX], bf16, tag="xout", name="xout")
                    for c in range(NCH):
                        xop = xps.tile([128, CHN], fp32, tag="xo", name="xop")
                        for cpc in range(4):
                            nc.tensor.matmul(
                                xop,
                                to_sb[:, cpc, :],
                                pr2[:, cpc * PIX + c * CHN:cpc * PIX + (c + 1) * CHN],
                                start=(cpc == 0),
                                stop=(cpc == 3),
                            )
                        nc.scalar.copy(xout[:, c * CHN:(c + 1) * CHN], xop[0:64, :])
                        if g == G - 1 and o == TO - 1:
                            nc.sync.dma_start(
                                out=out_r[o, g, :, c * CHN:(c + 1) * CHN],
                                in_=xout[:, c * CHN:(c + 1) * CHN],
                            )
                    if not (g == G - 1 and o == TO - 1):
                        nc.sync.dma_start(out=out_r[o, g, :, :], in_=xout)


def _get_module():
    if "nc" not in _CACHE:
        _CACHE["nc"] = _build_module()
    return _CACHE["nc"]


def _consts(conv_w, conv_b):
    conv_w = np.asarray(conv_w, np.float32)
    conv_b = np.asarray(conv_b, np.float32)
    # fused conv lhsT per (group-pair, reordered corr-row r'): r' = tt*9 +
    # grp*3 + k covers (dj, di) = (DJ_GROUPS[grp], k-1), i.e. original column
    # ij = k*3 + DJ_GROUPS[grp] + 1.  wf[kk=gh*64+cp, gp, r', m] with
    # m = gh*64 + o*8 + t; weight independent of cp, block-diagonal in gh.
    # Bias (+1.0 residual when t==o) applied at the PSUM drain (bvec).
    wf = np.zeros((128, 2, 63, 128), np.float32)
    bv = np.zeros((128, 2), np.float32)
    rmap = np.zeros(63, np.int64)
    for tt in range(7):
        for grp, dj in enumerate(DJ_GROUPS):
            for k in range(3):
                rmap[tt * 9 + grp * 3 + k] = tt * 9 + k * 3 + dj + 1
    for gp in range(2):
        for gh in range(2):
            g = gp * 2 + gh
            for m in range(64):
                wf[gh * 64:(gh + 1) * 64, gp, :, gh * 64 + m] = conv_w[g, m][rmap][None, :]
                bv[gh * 64 + m, gp] = conv_b[g, m] + (1.0 if (m // 8) == (m % 8) else 0.0)

    # t-reduce ones: tones[p=(cpk,t), cpc, m] = 1 iff m == cpc*16 + cpk
    to = np.zeros((128, 4, 128), np.float32)
    for cpc in range(4):
        for cpk in range(16):
            to[cpk * 8:(cpk + 1) * 8, cpc, cpc * 16 + cpk] = 1.0

    return (
        wf.astype(ml_dtypes.bfloat16),
        to.astype(ml_dtypes.bfloat16),
        bv,
    )


def kernel(x, conv_w, conv_b):
    from concourse.bass_utils import run_bass_kernel_spmd

    nc = _get_module()
    wf, to, bv = _consts(conv_w, conv_b)
    x = np.asarray(x, np.float32).astype(ml_dtypes.bfloat16)
    x8 = np.ascontiguousarray(x.reshape(NCORES, T, C, H, W))
    in_maps = [
        {
            "xin": np.ascontiguousarray(x8[i]),
            "wfin": wf,
            "tones": to,
            "bvec": bv,
        }
        for i in range(NCORES)
    ]
    res = run_bass_kernel_spmd(nc, in_maps, core_ids=list(range(NCORES)))
    outs = [r["out"] for r in res.results]
    return np.concatenate(outs, axis=0).astype(np.float32)
